# revision 8
# baseline (speedup 1.0000x reference)
"""MHSA block (b=8, c=256, h=w=32, nh=8) on 8 Trainium2 cores.

Sharding: pure data parallel -- one batch element per NeuronCore, no
collectives.  Per-core algorithm (X = x[b] as (C=256, L=1024)):

  QK   = Wqk @ X + bqk                       (512, L)   [q rows 0:256, k 256:512]
  V^T  = X^T @ Wv^T + bv  (head-padded)      (L, 264)   col h*33+32 == 1.0 (ones col)
  S^T_h = K_h^T Q_h  (per head, j-partition) (L, L)     -> exp(scale*S^T) = P^T
  [O_h; l_h] = V_aug,h^T.T @ P^T_h           (33, L)    row 32 = softmax denominators
  O_norm = O * replicate(1/l)                           (PE K=2 matmul replication)
  out  = x + Wproj @ O_norm + bproj

All matmuls run as float32r (full PE rate at free-dim >= 256).  exp() on
ScalarE is the per-core bottleneck (~8.4M elements).  S^T matmuls for two
heads share one PE pass via 32-row groups (tile_position auto-derived).
"""

import sys
import os

sys.path.insert(0, "/opt/trn_rl_repo")

from contextlib import ExitStack

import numpy as np

NH, DH, C, L = 8, 32, 256, 1024
B = 8
SCALE = DH ** -0.5
N_CORES = 8

_CACHE = {}


def _build_nc():
    import concourse.tile as tile
    from concourse import bacc, mybir

    f32 = mybir.dt.float32
    f32r = mybir.dt.float32r
    Exp = mybir.ActivationFunctionType.Exp

    nc = bacc.Bacc("TRN2", target_bir_lowering=False, debug=False)

    x_d = nc.dram_tensor("x", [C, L], f32r, kind="ExternalInput").ap()
    wqkT_d = nc.dram_tensor("wqkT", [C, 512], f32r, kind="ExternalInput").ap()
    bqk_d = nc.dram_tensor("bqk", [1, 512], f32r, kind="ExternalInput").ap()
    wvT_d = nc.dram_tensor("wvT", [C, 264], f32r, kind="ExternalInput").ap()
    bv_d = nc.dram_tensor("bv", [1, 264], f32r, kind="ExternalInput").ap()
    wpT_d = nc.dram_tensor("wpT", [512, 256], f32r, kind="ExternalInput").ap()
    bp_d = nc.dram_tensor("bp", [1, 256], f32r, kind="ExternalInput").ap()
    e_d = nc.dram_tensor("ee", [2, 128], f32r, kind="ExternalInput").ap()
    ones_d = nc.dram_tensor("ones_in", [1, 512], f32r, kind="ExternalInput").ap()
    out_d = nc.dram_tensor("out", [C, L], f32, kind="ExternalOutput").ap()

    def r(ap):
        return ap

    with tile.TileContext(nc) as tc, ExitStack() as ctx:
        persist = ctx.enter_context(tc.tile_pool(name="persist", bufs=1))
        ptpool = ctx.enter_context(tc.tile_pool(name="pt", bufs=3))
        onpool = ctx.enter_context(tc.tile_pool(name="on", bufs=2))
        smallp = ctx.enter_context(tc.tile_pool(name="small", bufs=2))
        qkv_ctx = ExitStack()
        qkps = qkv_ctx.enter_context(tc.tile_pool(name="qkps", bufs=2, space="PSUM"))
        vtps = qkv_ctx.enter_context(tc.tile_pool(name="vtps", bufs=2, space="PSUM"))

        # ---- constants / inputs to SBUF ----
        ones = persist.tile([1, 512], f32r, tag="ones", name="ones")
        nc.sync.dma_start(ones[:], ones_d[:])

        # warm the ACT exp table while QKV phase runs
        warm = persist.tile([1, 8], f32, tag="warm", name="warm")
        nc.gpsimd.memset(warm[:], 0.0)
        nc.scalar.activation(warm[:], warm[:], Exp)

        x_sb = []
        for t in range(2):
            xt = persist.tile([128, L], f32r, tag=f"x{t}", name=f"x{t}")
            nc.sync.dma_start(xt[:], x_d[t * 128:(t + 1) * 128, :])
            x_sb.append(xt)

        wqk_sb = []
        for t in range(2):
            w = persist.tile([128, 512], f32r, tag=f"wqk{t}", name=f"wqk{t}")
            nc.sync.dma_start(w[:], wqkT_d[t * 128:(t + 1) * 128, :])
            wqk_sb.append(w)
        bqk_sb = persist.tile([1, 512], f32r, tag="bqk", name="bqk")
        nc.sync.dma_start(bqk_sb[:], bqk_d[:])

        wv_sb = []
        for t in range(2):
            w = persist.tile([128, 264], f32r, tag=f"wv{t}", name=f"wv{t}")
            nc.sync.dma_start(w[:], wvT_d[t * 128:(t + 1) * 128, :])
            wv_sb.append(w)
        bv_sb = persist.tile([1, 264], f32r, tag="bv", name="bv")
        nc.sync.dma_start(bv_sb[:], bv_d[:])

        wp_sb = []
        for p in range(4):
            w = persist.tile([128, 256], f32r, tag=f"wp{p}", name=f"wp{p}")
            nc.sync.dma_start(w[:], wpT_d[p * 128:(p + 1) * 128, :])
            wp_sb.append(w)
        bp_sb = persist.tile([1, 256], f32r, tag="bp", name="bp")
        nc.sync.dma_start(bp_sb[:], bp_d[:])

        e_sb = persist.tile([2, 128], f32r, tag="ee", name="ee")
        nc.sync.dma_start(e_sb[:], e_d[:])

        o_sb = []
        for p in range(4):
            o = persist.tile([128, L], f32, tag=f"o{p}", name=f"o{p}")
            nc.gpsimd.memset(o[:], 0.0)
            o_sb.append(o)

        acc = [persist.tile([128, L], f32, tag=f"acc{t}", name=f"acc{t}") for t in range(2)]

        # ---- QK gemm:  QK(512, L) = WqkT.T @ X + bqk ----
        # emit q-heads-0-3 (mt 0), k-heads-0-3 (mt 2) first so attention on
        # pair 0/1 can begin before the rest of the QKV phase finishes.
        qk_sb = [None] * 4

        def qk_chunk(mt):
            ps = qkps.tile([128, L], f32, tag="qkps", name="qkps")
            for nh_ in range(2):
                o = ps[:, nh_ * 512:(nh_ + 1) * 512]
                for kt in range(2):
                    nc.tensor.matmul(
                        o,
                        lhsT=r(wqk_sb[kt][:, mt * 128:(mt + 1) * 128]),
                        rhs=r(x_sb[kt][:, nh_ * 512:(nh_ + 1) * 512]),
                        start=(kt == 0),
                        stop=False,
                    )
                nc.tensor.matmul(
                    o,
                    lhsT=r(bqk_sb[0:1, mt * 128:(mt + 1) * 128]),
                    rhs=r(ones[0:1, :]),
                    start=False,
                    stop=True,
                )
            qk = persist.tile([128, L], f32r, tag=f"qk{mt}", name=f"qk{mt}")
            nc.vector.tensor_copy(qk[:], ps[:])
            qk_sb[mt] = qk

        # ---- V^T gemm: VT(L, 264) = X.T @ WvT + bv  (head-padded cols) ----
        vt_sb = [None] * 8

        def vt_chunk(jt):
            ps = vtps.tile([128, 264], f32, tag="vtps", name="vtps")
            for kt in range(2):
                nc.tensor.matmul(
                    ps[:],
                    lhsT=r(x_sb[kt][:, jt * 128:(jt + 1) * 128]),
                    rhs=r(wv_sb[kt][:]),
                    start=(kt == 0),
                    stop=False,
                )
            nc.tensor.matmul(
                ps[:],
                lhsT=r(ones[0:1, 0:128]),
                rhs=r(bv_sb[0:1, :]),
                start=False,
                stop=True,
            )
            vt = persist.tile([128, 264], f32r, tag=f"vt{jt}", name=f"vt{jt}")
            nc.vector.tensor_copy(vt[:], ps[:])
            vt_sb[jt] = vt

        qk_chunk(0)
        qk_chunk(2)
        for jt in range(8):
            vt_chunk(jt)
        qk_chunk(1)
        qk_chunk(3)
        qkv_ctx.close()

        stps = ctx.enter_context(tc.tile_pool(name="stps", bufs=2, space="PSUM"))
        pvps = ctx.enter_context(tc.tile_pool(name="pvps", bufs=1, space="PSUM"))
        rpps = ctx.enter_context(tc.tile_pool(name="rpps", bufs=1, space="PSUM"))
        pjps = ctx.enter_context(tc.tile_pool(name="pjps", bufs=1, space="PSUM"))

        # ---- attention + per-pair normalization + proj partial ----
        for p in range(4):
            qt = qk_sb[p // 2]
            kt_ = qk_sb[2 + p // 2]
            oA = 64 * (p % 2)
            oB = oA + 32
            hA, hB = 2 * p, 2 * p + 1
            for ih in range(2):
                pvA = pvps.tile([33, 512], f32, tag="pvA", name="pvA")
                pvB = pvps.tile([33, 512], f32, tag="pvB", name="pvB")
                for jc in range(8):
                    st = stps.tile([128, L], f32, tag="st", name="st")
                    nc.tensor.matmul(
                        st[:, 0:512],
                        lhsT=r(kt_[oA:oA + 32, jc * 128:(jc + 1) * 128]),
                        rhs=r(qt[oA:oA + 32, ih * 512:(ih + 1) * 512]),
                        start=True,
                        stop=True,
                        tile_position=(oA, 0),
                    )
                    nc.tensor.matmul(
                        st[:, 512:1024],
                        lhsT=r(kt_[oB:oB + 32, jc * 128:(jc + 1) * 128]),
                        rhs=r(qt[oB:oB + 32, ih * 512:(ih + 1) * 512]),
                        start=True,
                        stop=True,
                        tile_position=(oB, 0),
                    )
                    pt = ptpool.tile([128, L], f32r, tag="pt", name="pt")
                    nc.scalar.activation(pt[:], st[:], Exp, scale=SCALE)
                    nc.tensor.matmul(
                        pvA[:],
                        lhsT=r(vt_sb[jc][:, hA * 33:hA * 33 + 33]),
                        rhs=r(pt[:, 0:512]),
                        start=(jc == 0),
                        stop=(jc == 7),
                    )
                    nc.tensor.matmul(
                        pvB[:],
                        lhsT=r(vt_sb[jc][:, hB * 33:hB * 33 + 33]),
                        rhs=r(pt[:, 512:1024]),
                        start=(jc == 0),
                        stop=(jc == 7),
                    )
                nc.vector.tensor_copy(o_sb[p][0:33, ih * 512:(ih + 1) * 512], pvA[:])
                nc.vector.tensor_copy(o_sb[p][64:97, ih * 512:(ih + 1) * 512], pvB[:])

            # l rows sit at partitions 32 (head A) and 96 (head B)
            l_sb = smallp.tile([2, L], f32, tag="l", name="l")
            nc.sync.dma_start(l_sb[0:1, :], o_sb[p][32:33, :])
            nc.sync.dma_start(l_sb[1:2, :], o_sb[p][96:97, :])
            rl32 = smallp.tile([2, L], f32, tag="rl32", name="rl32")
            scr = smallp.tile([2, L], f32, tag="rlscratch", name="rlscratch")
            nc.vector.reciprocal_approx_accurate(rl32[:], l_sb[:], scr[:])
            rl = smallp.tile([2, L], f32r, tag="rl", name="rl")
            nc.vector.tensor_copy(rl[:], rl32[:])

            for nh_ in range(2):
                rp = rpps.tile([128, 512], f32, tag="rp", name="rp")
                nc.tensor.matmul(
                    rp[:],
                    lhsT=r(e_sb[:]),
                    rhs=r(rl[:, nh_ * 512:(nh_ + 1) * 512]),
                    start=True,
                    stop=True,
                )
                on = onpool.tile([128, 512], f32r, tag="on", name="on")
                nc.vector.tensor_mul(on[:], o_sb[p][:, nh_ * 512:(nh_ + 1) * 512], rp[:])
                for mt2 in range(2):
                    pj = pjps.tile([128, 512], f32, tag="pj", name="pj")
                    nc.tensor.matmul(
                        pj[:],
                        lhsT=r(wp_sb[p][:, mt2 * 128:(mt2 + 1) * 128]),
                        rhs=r(on[:]),
                        start=True,
                        stop=(p != 0),
                    )
                    if p == 0:
                        nc.tensor.matmul(
                            pj[:],
                            lhsT=r(bp_sb[0:1, mt2 * 128:(mt2 + 1) * 128]),
                            rhs=r(ones[0:1, :]),
                            start=False,
                            stop=True,
                        )
                        nc.vector.tensor_add(
                            acc[mt2][:, nh_ * 512:(nh_ + 1) * 512],
                            x_sb[mt2][:, nh_ * 512:(nh_ + 1) * 512].bitcast(f32),
                            pj[:],
                        )
                    else:
                        nc.vector.tensor_add(
                            acc[mt2][:, nh_ * 512:(nh_ + 1) * 512],
                            acc[mt2][:, nh_ * 512:(nh_ + 1) * 512],
                            pj[:],
                        )

        for mt2 in range(2):
            nc.sync.dma_start(out_d[mt2 * 128:(mt2 + 1) * 128, :], acc[mt2][:])

    nc.compile()
    return nc


def _get_nc():
    if "nc" not in _CACHE:
        _CACHE["nc"] = _build_nc()
    return _CACHE["nc"]


def _pack_weights(w_qkv, b_qkv, w_proj, b_proj):
    w_qkv = np.asarray(w_qkv, dtype=np.float32)
    b_qkv = np.asarray(b_qkv, dtype=np.float32)
    w_proj = np.asarray(w_proj, dtype=np.float32)
    b_proj = np.asarray(b_proj, dtype=np.float32)

    wqkT = np.ascontiguousarray(w_qkv[:512].T)          # (256, 512)
    bqk = np.ascontiguousarray(b_qkv[:512].reshape(1, 512))

    wvT = np.zeros((C, 264), dtype=np.float32)
    bv = np.zeros((1, 264), dtype=np.float32)
    for h in range(NH):
        wvT[:, h * 33:h * 33 + 32] = w_qkv[512 + h * 32:512 + (h + 1) * 32].T
        bv[0, h * 33:h * 33 + 32] = b_qkv[512 + h * 32:512 + (h + 1) * 32]
        bv[0, h * 33 + 32] = 1.0

    # o_sb row layout per pair tile p: head 2p at rows 0:32 (l at 32),
    # head 2p+1 at rows 64:96 (l at 96); all other rows zero.
    wpT = np.zeros((512, 256), dtype=np.float32)
    for p in range(4):
        wpT[p * 128 + 0:p * 128 + 32, :] = w_proj[:, (2 * p) * 32:(2 * p + 1) * 32].T
        wpT[p * 128 + 64:p * 128 + 96, :] = w_proj[:, (2 * p + 1) * 32:(2 * p + 2) * 32].T
    bp = np.ascontiguousarray(b_proj.reshape(1, 256))

    ee = np.zeros((2, 128), dtype=np.float32)
    ee[0, 0:32] = 1.0
    ee[1, 64:96] = 1.0
    ones_in = np.ones((1, 512), dtype=np.float32)
    return dict(wqkT=wqkT, bqk=bqk, wvT=wvT, bv=bv, wpT=wpT, bp=bp, ee=ee,
                ones_in=ones_in)


def _install_ntff_hook_module():
    """bass_utils wants antenv.axon_hooks for trace=True under axon; this
    image's antenv lacks it.  Inject an equivalent module into sys.modules."""
    if "antenv.axon_hooks" in sys.modules:
        return
    try:
        import antenv.axon_hooks  # noqa: F401

        return
    except ImportError:
        pass
    import contextlib
    import ctypes
    import types

    mod = types.ModuleType("antenv.axon_hooks")
    state = {"hook": None, "inited": False}

    def _default_hook():
        so_path = "/opt/axon/libaxon_pjrt.so"
        if not os.path.exists(so_path):
            return None
        lib = ctypes.CDLL(so_path)
        if not hasattr(lib, "axon_start_nrt_profile"):
            return None
        lib.axon_start_nrt_profile.argtypes = [
            ctypes.POINTER(ctypes.c_int64),
            ctypes.c_size_t,
        ]
        lib.axon_start_nrt_profile.restype = ctypes.c_int64
        lib.axon_stop_nrt_profile.argtypes = [ctypes.c_char_p]
        lib.axon_stop_nrt_profile.restype = ctypes.c_int64

        @contextlib.contextmanager
        def _hook(output_dir, device_ids):
            import jax

            jax.devices()
            if device_ids:
                ids = (ctypes.c_int64 * len(device_ids))(*device_ids)
                rc = lib.axon_start_nrt_profile(ids, len(device_ids))
            else:
                rc = lib.axon_start_nrt_profile(None, 0)
            if rc != 0:
                raise RuntimeError(f"axon_start_nrt_profile rc={rc}")
            try:
                yield
            finally:
                n = lib.axon_stop_nrt_profile(str(output_dir).encode())
                if n < 0:
                    raise RuntimeError(f"axon_stop_nrt_profile rc={n}")
                print(f"profile: {n} file(s) written to {output_dir}")

        return _hook

    def set_axon_ntff_profile_hook(hook):
        state["hook"] = hook
        state["inited"] = True

    def get_axon_ntff_profile_hook():
        if not state["inited"]:
            state["hook"] = _default_hook()
            state["inited"] = True
        return state["hook"]

    mod.set_axon_ntff_profile_hook = set_axon_ntff_profile_hook
    mod.get_axon_ntff_profile_hook = get_axon_ntff_profile_hook
    sys.modules["antenv.axon_hooks"] = mod


def kernel(x, w_qkv, b_qkv, w_proj, b_proj, _trace=False, _trace_kwargs=None):
    if _trace:
        _install_ntff_hook_module()
    from concourse.bass_utils import run_bass_kernel_spmd

    x = np.asarray(x, dtype=np.float32)
    b, c, h, w = x.shape
    assert (b, c, h, w) == (B, C, 32, 32)

    weights = _pack_weights(w_qkv, b_qkv, w_proj, b_proj)
    nc = _get_nc()

    in_maps = []
    for core in range(N_CORES):
        m = dict(weights)
        m["x"] = np.ascontiguousarray(x[core].reshape(C, L))
        in_maps.append(m)

    res = run_bass_kernel_spmd(
        nc,
        in_maps,
        list(range(N_CORES)),
        trace=_trace,
        **(_trace_kwargs or {}),
    )
    out = np.stack([res.results[core]["out"] for core in range(N_CORES)])
    if _trace:
        _CACHE["last_result"] = res
    return out.reshape(B, C, 32, 32)


# revision 9
# speedup vs baseline: 1.1271x; 1.1271x over previous
"""MHSA block (b=8, c=256, h=w=32, nh=8) on 8 Trainium2 cores.

Sharding: pure data parallel -- one batch element per NeuronCore, no
collectives.  Per-core algorithm (X = x[b] as (C=256, L=1024)):

  QK   = Wqk @ X + bqk                       (512, L)   [q rows 0:256, k 256:512]
  V^T  = X^T @ Wv^T + bv  (head-padded)      (L, 264)   col h*33+32 == 1.0 (ones col)
  S^T_h = K_h^T Q_h  (per head, j-partition) (L, L)     -> exp(scale*S^T) = P^T
  [O_h; l_h] = V_aug,h^T.T @ P^T_h           (33, L)    row 32 = softmax denominators
  O_norm = O * replicate(1/l)                           (PE K=2 matmul replication)
  out  = x + Wproj @ O_norm + bproj

All matmuls run as float32r (full PE rate at free-dim >= 256).  exp() on
ScalarE is the per-core bottleneck (~8.4M elements).  S^T matmuls for two
heads share one PE pass via 32-row groups (tile_position auto-derived).
"""

import sys
import os

sys.path.insert(0, "/opt/trn_rl_repo")

from contextlib import ExitStack

import numpy as np

NH, DH, C, L = 8, 32, 256, 1024
B = 8
SCALE = DH ** -0.5
N_CORES = 8

_CACHE = {}


def _build_nc():
    import concourse.tile as tile
    from concourse import bacc, mybir

    f32 = mybir.dt.float32
    f32r = mybir.dt.float32r
    bf16 = mybir.dt.bfloat16
    Exp = mybir.ActivationFunctionType.Exp

    nc = bacc.Bacc("TRN2", target_bir_lowering=False, debug=False)

    x_d = nc.dram_tensor("x", [C, L], bf16, kind="ExternalInput").ap()
    xf_d = nc.dram_tensor("xf", [C, L], f32, kind="ExternalInput").ap()
    wqkT_d = nc.dram_tensor("wqkT", [C, 512], bf16, kind="ExternalInput").ap()
    bqk_d = nc.dram_tensor("bqk", [1, 512], bf16, kind="ExternalInput").ap()
    wvT_d = nc.dram_tensor("wvT", [C, 264], bf16, kind="ExternalInput").ap()
    bv_d = nc.dram_tensor("bv", [1, 264], bf16, kind="ExternalInput").ap()
    wpT_d = nc.dram_tensor("wpT", [512, 256], bf16, kind="ExternalInput").ap()
    bp_d = nc.dram_tensor("bp", [1, 256], bf16, kind="ExternalInput").ap()
    e_d = nc.dram_tensor("ee", [2, 128], bf16, kind="ExternalInput").ap()
    ones_d = nc.dram_tensor("ones_in", [1, 512], bf16, kind="ExternalInput").ap()
    out_d = nc.dram_tensor("out", [C, L], f32, kind="ExternalOutput").ap()

    def r(ap):
        return ap

    with tile.TileContext(nc) as tc, ExitStack() as ctx:
        persist = ctx.enter_context(tc.tile_pool(name="persist", bufs=1))
        ptpool = ctx.enter_context(tc.tile_pool(name="pt", bufs=3))
        onpool = ctx.enter_context(tc.tile_pool(name="on", bufs=2))
        smallp = ctx.enter_context(tc.tile_pool(name="small", bufs=2))
        qkv_ctx = ExitStack()
        qkps = qkv_ctx.enter_context(tc.tile_pool(name="qkps", bufs=2, space="PSUM"))
        vtps = qkv_ctx.enter_context(tc.tile_pool(name="vtps", bufs=2, space="PSUM"))

        # ---- constants / inputs to SBUF ----
        ones = persist.tile([1, 512], bf16, tag="ones", name="ones")
        nc.sync.dma_start(ones[:], ones_d[:])

        # warm the ACT exp table while QKV phase runs
        warm = persist.tile([1, 8], f32, tag="warm", name="warm")
        nc.gpsimd.memset(warm[:], 0.0)
        nc.scalar.activation(warm[:], warm[:], Exp)

        x_sb = []
        xf_sb = []
        for t in range(2):
            xt = persist.tile([128, L], bf16, tag=f"x{t}", name=f"x{t}")
            nc.sync.dma_start(xt[:], x_d[t * 128:(t + 1) * 128, :])
            x_sb.append(xt)
            xft = persist.tile([128, L], f32, tag=f"xf{t}", name=f"xf{t}")
            nc.sync.dma_start(xft[:], xf_d[t * 128:(t + 1) * 128, :])
            xf_sb.append(xft)

        wqk_sb = []
        for t in range(2):
            w = persist.tile([128, 512], bf16, tag=f"wqk{t}", name=f"wqk{t}")
            nc.sync.dma_start(w[:], wqkT_d[t * 128:(t + 1) * 128, :])
            wqk_sb.append(w)
        bqk_sb = persist.tile([1, 512], bf16, tag="bqk", name="bqk")
        nc.sync.dma_start(bqk_sb[:], bqk_d[:])

        wv_sb = []
        for t in range(2):
            w = persist.tile([128, 264], bf16, tag=f"wv{t}", name=f"wv{t}")
            nc.sync.dma_start(w[:], wvT_d[t * 128:(t + 1) * 128, :])
            wv_sb.append(w)
        bv_sb = persist.tile([1, 264], bf16, tag="bv", name="bv")
        nc.sync.dma_start(bv_sb[:], bv_d[:])

        wp_sb = []
        for p in range(4):
            w = persist.tile([128, 256], bf16, tag=f"wp{p}", name=f"wp{p}")
            nc.sync.dma_start(w[:], wpT_d[p * 128:(p + 1) * 128, :])
            wp_sb.append(w)
        bp_sb = persist.tile([1, 256], bf16, tag="bp", name="bp")
        nc.sync.dma_start(bp_sb[:], bp_d[:])

        e_sb = persist.tile([2, 128], bf16, tag="ee", name="ee")
        nc.sync.dma_start(e_sb[:], e_d[:])

        o_sb = []
        for p in range(4):
            o = persist.tile([128, L], f32, tag=f"o{p}", name=f"o{p}")
            nc.gpsimd.memset(o[:], 0.0)
            o_sb.append(o)

        acc = [persist.tile([128, L], f32, tag=f"acc{t}", name=f"acc{t}") for t in range(2)]

        # ---- QK gemm:  QK(512, L) = WqkT.T @ X + bqk ----
        # emit q-heads-0-3 (mt 0), k-heads-0-3 (mt 2) first so attention on
        # pair 0/1 can begin before the rest of the QKV phase finishes.
        qk_sb = [None] * 4

        def qk_chunk(mt):
            ps = qkps.tile([128, L], f32, tag="qkps", name="qkps")
            for nh_ in range(2):
                o = ps[:, nh_ * 512:(nh_ + 1) * 512]
                for kt in range(2):
                    nc.tensor.matmul(
                        o,
                        lhsT=r(wqk_sb[kt][:, mt * 128:(mt + 1) * 128]),
                        rhs=r(x_sb[kt][:, nh_ * 512:(nh_ + 1) * 512]),
                        start=(kt == 0),
                        stop=False,
                    )
                nc.tensor.matmul(
                    o,
                    lhsT=r(bqk_sb[0:1, mt * 128:(mt + 1) * 128]),
                    rhs=r(ones[0:1, :]),
                    start=False,
                    stop=True,
                )
            qk = persist.tile([128, L], bf16, tag=f"qk{mt}", name=f"qk{mt}")
            nc.vector.tensor_copy(qk[:], ps[:])
            qk_sb[mt] = qk

        # ---- V^T gemm: VT(L, 264) = X.T @ WvT + bv  (head-padded cols) ----
        vt_sb = [None] * 8

        def vt_chunk(jt):
            ps = vtps.tile([128, 264], f32, tag="vtps", name="vtps")
            for kt in range(2):
                nc.tensor.matmul(
                    ps[:],
                    lhsT=r(x_sb[kt][:, jt * 128:(jt + 1) * 128]),
                    rhs=r(wv_sb[kt][:]),
                    start=(kt == 0),
                    stop=False,
                )
            nc.tensor.matmul(
                ps[:],
                lhsT=r(ones[0:1, 0:128]),
                rhs=r(bv_sb[0:1, :]),
                start=False,
                stop=True,
            )
            vt = persist.tile([128, 264], bf16, tag=f"vt{jt}", name=f"vt{jt}")
            nc.vector.tensor_copy(vt[:], ps[:])
            vt_sb[jt] = vt

        qk_chunk(0)
        qk_chunk(2)
        for jt in range(8):
            vt_chunk(jt)
        qk_chunk(1)
        qk_chunk(3)
        qkv_ctx.close()

        stps = ctx.enter_context(tc.tile_pool(name="stps", bufs=2, space="PSUM"))
        pvps = ctx.enter_context(tc.tile_pool(name="pvps", bufs=1, space="PSUM"))
        rpps = ctx.enter_context(tc.tile_pool(name="rpps", bufs=1, space="PSUM"))
        pjps = ctx.enter_context(tc.tile_pool(name="pjps", bufs=1, space="PSUM"))

        # ---- attention + per-pair normalization + proj partial ----
        for p in range(4):
            qt = qk_sb[p // 2]
            kt_ = qk_sb[2 + p // 2]
            oA = 64 * (p % 2)
            oB = oA + 32
            hA, hB = 2 * p, 2 * p + 1
            for ih in range(2):
                pvA = pvps.tile([33, 512], f32, tag="pvA", name="pvA")
                pvB = pvps.tile([33, 512], f32, tag="pvB", name="pvB")
                for jc in range(8):
                    st = stps.tile([128, L], f32, tag="st", name="st")
                    nc.tensor.matmul(
                        st[:, 0:512],
                        lhsT=r(kt_[oA:oA + 32, jc * 128:(jc + 1) * 128]),
                        rhs=r(qt[oA:oA + 32, ih * 512:(ih + 1) * 512]),
                        start=True,
                        stop=True,
                        tile_position=(oA, 0),
                    )
                    nc.tensor.matmul(
                        st[:, 512:1024],
                        lhsT=r(kt_[oB:oB + 32, jc * 128:(jc + 1) * 128]),
                        rhs=r(qt[oB:oB + 32, ih * 512:(ih + 1) * 512]),
                        start=True,
                        stop=True,
                        tile_position=(oB, 0),
                    )
                    pt = ptpool.tile([128, L], bf16, tag="pt", name="pt")
                    nc.scalar.activation(pt[:], st[:], Exp, scale=SCALE)
                    nc.tensor.matmul(
                        pvA[:],
                        lhsT=r(vt_sb[jc][:, hA * 33:hA * 33 + 33]),
                        rhs=r(pt[:, 0:512]),
                        start=(jc == 0),
                        stop=(jc == 7),
                    )
                    nc.tensor.matmul(
                        pvB[:],
                        lhsT=r(vt_sb[jc][:, hB * 33:hB * 33 + 33]),
                        rhs=r(pt[:, 512:1024]),
                        start=(jc == 0),
                        stop=(jc == 7),
                    )
                nc.vector.tensor_copy(o_sb[p][0:33, ih * 512:(ih + 1) * 512], pvA[:])
                nc.vector.tensor_copy(o_sb[p][64:97, ih * 512:(ih + 1) * 512], pvB[:])

            # l rows sit at partitions 32 (head A) and 96 (head B)
            l_sb = smallp.tile([2, L], f32, tag="l", name="l")
            nc.sync.dma_start(l_sb[0:1, :], o_sb[p][32:33, :])
            nc.sync.dma_start(l_sb[1:2, :], o_sb[p][96:97, :])
            rl32 = smallp.tile([2, L], f32, tag="rl32", name="rl32")
            scr = smallp.tile([2, L], f32, tag="rlscratch", name="rlscratch")
            nc.vector.reciprocal_approx_accurate(rl32[:], l_sb[:], scr[:])
            rl = smallp.tile([2, L], bf16, tag="rl", name="rl")
            nc.vector.tensor_copy(rl[:], rl32[:])

            for nh_ in range(2):
                rp = rpps.tile([128, 512], f32, tag="rp", name="rp")
                nc.tensor.matmul(
                    rp[:],
                    lhsT=r(e_sb[:]),
                    rhs=r(rl[:, nh_ * 512:(nh_ + 1) * 512]),
                    start=True,
                    stop=True,
                )
                on = onpool.tile([128, 512], bf16, tag="on", name="on")
                nc.vector.tensor_mul(on[:], o_sb[p][:, nh_ * 512:(nh_ + 1) * 512], rp[:])
                for mt2 in range(2):
                    pj = pjps.tile([128, 512], f32, tag="pj", name="pj")
                    nc.tensor.matmul(
                        pj[:],
                        lhsT=r(wp_sb[p][:, mt2 * 128:(mt2 + 1) * 128]),
                        rhs=r(on[:]),
                        start=True,
                        stop=(p != 0),
                    )
                    if p == 0:
                        nc.tensor.matmul(
                            pj[:],
                            lhsT=r(bp_sb[0:1, mt2 * 128:(mt2 + 1) * 128]),
                            rhs=r(ones[0:1, :]),
                            start=False,
                            stop=True,
                        )
                        nc.vector.tensor_add(
                            acc[mt2][:, nh_ * 512:(nh_ + 1) * 512],
                            xf_sb[mt2][:, nh_ * 512:(nh_ + 1) * 512],
                            pj[:],
                        )
                    else:
                        nc.vector.tensor_add(
                            acc[mt2][:, nh_ * 512:(nh_ + 1) * 512],
                            acc[mt2][:, nh_ * 512:(nh_ + 1) * 512],
                            pj[:],
                        )

        for mt2 in range(2):
            nc.sync.dma_start(out_d[mt2 * 128:(mt2 + 1) * 128, :], acc[mt2][:])

    nc.compile()
    return nc


def _get_nc():
    if "nc" not in _CACHE:
        _CACHE["nc"] = _build_nc()
    return _CACHE["nc"]


def _pack_weights(w_qkv, b_qkv, w_proj, b_proj):
    w_qkv = np.asarray(w_qkv, dtype=np.float32)
    b_qkv = np.asarray(b_qkv, dtype=np.float32)
    w_proj = np.asarray(w_proj, dtype=np.float32)
    b_proj = np.asarray(b_proj, dtype=np.float32)

    wqkT = np.ascontiguousarray(w_qkv[:512].T)          # (256, 512)
    bqk = np.ascontiguousarray(b_qkv[:512].reshape(1, 512))

    wvT = np.zeros((C, 264), dtype=np.float32)
    bv = np.zeros((1, 264), dtype=np.float32)
    for h in range(NH):
        wvT[:, h * 33:h * 33 + 32] = w_qkv[512 + h * 32:512 + (h + 1) * 32].T
        bv[0, h * 33:h * 33 + 32] = b_qkv[512 + h * 32:512 + (h + 1) * 32]
        bv[0, h * 33 + 32] = 1.0

    # o_sb row layout per pair tile p: head 2p at rows 0:32 (l at 32),
    # head 2p+1 at rows 64:96 (l at 96); all other rows zero.
    wpT = np.zeros((512, 256), dtype=np.float32)
    for p in range(4):
        wpT[p * 128 + 0:p * 128 + 32, :] = w_proj[:, (2 * p) * 32:(2 * p + 1) * 32].T
        wpT[p * 128 + 64:p * 128 + 96, :] = w_proj[:, (2 * p + 1) * 32:(2 * p + 2) * 32].T
    bp = np.ascontiguousarray(b_proj.reshape(1, 256))

    ee = np.zeros((2, 128), dtype=np.float32)
    ee[0, 0:32] = 1.0
    ee[1, 64:96] = 1.0
    ones_in = np.ones((1, 512), dtype=np.float32)
    return dict(wqkT=wqkT, bqk=bqk, wvT=wvT, bv=bv, wpT=wpT, bp=bp, ee=ee,
                ones_in=ones_in)


def _install_ntff_hook_module():
    """bass_utils wants antenv.axon_hooks for trace=True under axon; this
    image's antenv lacks it.  Inject an equivalent module into sys.modules."""
    if "antenv.axon_hooks" in sys.modules:
        return
    try:
        import antenv.axon_hooks  # noqa: F401

        return
    except ImportError:
        pass
    import contextlib
    import ctypes
    import types

    mod = types.ModuleType("antenv.axon_hooks")
    state = {"hook": None, "inited": False}

    def _default_hook():
        so_path = "/opt/axon/libaxon_pjrt.so"
        if not os.path.exists(so_path):
            return None
        lib = ctypes.CDLL(so_path)
        if not hasattr(lib, "axon_start_nrt_profile"):
            return None
        lib.axon_start_nrt_profile.argtypes = [
            ctypes.POINTER(ctypes.c_int64),
            ctypes.c_size_t,
        ]
        lib.axon_start_nrt_profile.restype = ctypes.c_int64
        lib.axon_stop_nrt_profile.argtypes = [ctypes.c_char_p]
        lib.axon_stop_nrt_profile.restype = ctypes.c_int64

        @contextlib.contextmanager
        def _hook(output_dir, device_ids):
            import jax

            jax.devices()
            if device_ids:
                ids = (ctypes.c_int64 * len(device_ids))(*device_ids)
                rc = lib.axon_start_nrt_profile(ids, len(device_ids))
            else:
                rc = lib.axon_start_nrt_profile(None, 0)
            if rc != 0:
                raise RuntimeError(f"axon_start_nrt_profile rc={rc}")
            try:
                yield
            finally:
                n = lib.axon_stop_nrt_profile(str(output_dir).encode())
                if n < 0:
                    raise RuntimeError(f"axon_stop_nrt_profile rc={n}")
                print(f"profile: {n} file(s) written to {output_dir}")

        return _hook

    def set_axon_ntff_profile_hook(hook):
        state["hook"] = hook
        state["inited"] = True

    def get_axon_ntff_profile_hook():
        if not state["inited"]:
            state["hook"] = _default_hook()
            state["inited"] = True
        return state["hook"]

    mod.set_axon_ntff_profile_hook = set_axon_ntff_profile_hook
    mod.get_axon_ntff_profile_hook = get_axon_ntff_profile_hook
    sys.modules["antenv.axon_hooks"] = mod


def _bf16(a):
    import ml_dtypes

    return np.asarray(a).astype(ml_dtypes.bfloat16)


def kernel(x, w_qkv, b_qkv, w_proj, b_proj, _trace=False, _trace_kwargs=None):
    if _trace:
        _install_ntff_hook_module()
    from concourse.bass_utils import run_bass_kernel_spmd

    x = np.asarray(x, dtype=np.float32)
    b, c, h, w = x.shape
    assert (b, c, h, w) == (B, C, 32, 32)

    weights = _pack_weights(w_qkv, b_qkv, w_proj, b_proj)
    nc = _get_nc()

    weights = {k: _bf16(v) for k, v in weights.items()}
    in_maps = []
    for core in range(N_CORES):
        m = dict(weights)
        xm = np.ascontiguousarray(x[core].reshape(C, L))
        m["x"] = _bf16(xm)
        m["xf"] = xm
        in_maps.append(m)

    res = run_bass_kernel_spmd(
        nc,
        in_maps,
        list(range(N_CORES)),
        trace=_trace,
        **(_trace_kwargs or {}),
    )
    out = np.stack([res.results[core]["out"] for core in range(N_CORES)])
    if _trace:
        _CACHE["last_result"] = res
    return out.reshape(B, C, 32, 32)
